# revision 5
# baseline (speedup 1.0000x reference)
"""Trainium2 Bass kernel for nn_ConcatAttention_Param.

Reference computation (per batch b):
    pre[l,h] = sum_i h[b,l,i] * W_h[h,i] + bias[h]     (W_h = ln_w[:, :I], bias = ln_b + W_vq @ vq)
    s[l]     = tanh(pre[l,:]) @ v_w
    s       += -10000 * (~mask[b,l])
    a        = softmax(s over l)
    r[b,:]   = sum_l a[l] * h[b,l,:]

Key optimizations over a dense implementation:
  * Mask compaction: masked positions get s-10000, whose exp underflows to
    exactly 0 in fp32, so they contribute nothing to the softmax or to r.
    The host gathers only the unmasked rows of h (~half of L). This halves
    PE work, DMA traffic and tanh work. A fully-masked batch keeps all rows
    with no mask-add (softmax(s-1e4) == softmax(s)).
  * Count-sorted slot assignment: batches sorted by unmasked count, rank
    8j+c -> core c slot j, so slot j's padded length is the octile max
    (32-elem granularity) instead of the global max.
  * bf16 operands on the PE (measured end-to-end rel err ~3e-3 vs 2e-2
    budget). bf16 enables fast-weight-load and tile_position col-packing,
    both rejected for fp32r. HW microbench: LDWEIGHTS is fully hidden, and
    4-wide col-packed M=1 matmuls run concurrently (70.9 ns/MM vs 264).
  * Score dots (M=1) run 4-wide col-packed, combined with a ones matmul.
  * Pass 2 (r = e @ h) runs 4-wide col-packed at N=256.
  * Flat software pipeline over (batch, group) units: packed-score and
    combine emission are deferred one unit (hiding ACT/DVE latency inside
    the next unit's matmul stream, across batch boundaries too); pass2(b)
    is emitted one batch late so the PE never waits on the softmax chain.

Data-parallel over batch: 4 batches per core x 8 cores.
"""

from contextlib import ExitStack

import numpy as np
import ml_dtypes

import jax


import concourse.bass as bass
import concourse.tile as tile
from concourse import bacc, mybir

# Problem constants (hardcoded per contract; kernel.py may not read spec.json)
B_FULL = 32
L = 2048
I = 1024
H = 1024
N_CORES = 8
B_PC = B_FULL // N_CORES  # batches per core

LG = 512            # max l-group (moving-operand columns per matmul)
P = 128             # partitions
IC = I // P         # i chunks
HC = H // P         # h' chunks
FR = mybir.dt.float32r
F32 = mybir.dt.float32
BF = mybir.dt.bfloat16
NPBF = ml_dtypes.bfloat16

MASK_PAD = -30000.0
GRAN = 32           # score-region padding granularity


def _groups(n):
    """Split n into moving-operand column groups (<=512, mult of GRAN)."""
    offs = []
    off = 0
    while off < n:
        g = min(LG, n - off)
        offs.append((off, g))
        off += g
    return offs


def _ceil(x, m):
    return (x + m - 1) // m * m


def build_module(slot_lens, b_pc: int = B_PC, static_reps: bool = False):
    """Build the per-core Bass module (same program on every core).

    slot_lens: per-batch-slot compacted lengths (mult of GRAN). The score
    region of slot j spans lens[j] columns; the softmax/pass-2 region is
    padded to a multiple of 128.
    """
    lens = list(slot_lens)
    assert len(lens) == b_pc and all(x % GRAN == 0 for x in lens)
    lens128 = [_ceil(x, P) for x in lens]
    offs = np.cumsum([0] + lens).tolist()       # hT / m_add column offsets
    offs128 = np.cumsum([0] + lens128).tolist()  # hN row offsets
    ltot, ltot128 = offs[-1], offs128[-1]

    nc = bacc.Bacc("TRN2", target_bir_lowering=False, debug=False,
                   enable_asserts=False, num_devices=N_CORES)

    hT_d = nc.dram_tensor("hT", (I, ltot), BF, kind="ExternalInput").ap()
    hN_d = nc.dram_tensor("hN", (ltot128, I), BF, kind="ExternalInput").ap()
    w_d = nc.dram_tensor("w_hT", (I, H), BF, kind="ExternalInput").ap()
    vw_d = nc.dram_tensor("v_w", (H,), BF, kind="ExternalInput").ap()
    bias_d = nc.dram_tensor("bias", (H,), F32, kind="ExternalInput").ap()
    madd_d = nc.dram_tensor("m_add", (1, ltot), F32, kind="ExternalInput").ap()
    ones_d = nc.dram_tensor("ones97", (97, 1), FR, kind="ExternalInput").ap()
    reps_d = nc.dram_tensor("reps", (1, 1), mybir.dt.int32,
                            kind="ExternalInput").ap()
    r_d = nc.dram_tensor("r", (b_pc, I), F32, kind="ExternalOutput").ap()

    with tile.TileContext(nc) as tc, ExitStack() as ctx:
        const_p = ctx.enter_context(tc.tile_pool(name="const", bufs=1))
        hT_p = ctx.enter_context(tc.tile_pool(name="hT", bufs=24))
        tanh_p = ctx.enter_context(tc.tile_pool(name="tanh", bufs=20))
        hN_p = ctx.enter_context(tc.tile_pool(name="hN", bufs=6))
        small_p = ctx.enter_context(tc.tile_pool(name="small", bufs=2))
        pre_ps = ctx.enter_context(tc.tile_pool(name="preps", bufs=3, space="PSUM"))
        s_ps = ctx.enter_context(tc.tile_pool(name="sps", bufs=2, space="PSUM"))
        comb_ps = ctx.enter_context(tc.tile_pool(name="combps", bufs=2, space="PSUM"))
        r_ps = ctx.enter_context(tc.tile_pool(name="rps", bufs=1, space="PSUM"))
        dram_p = ctx.enter_context(tc.tile_pool(name="edram", bufs=2, space="DRAM"))

        # --- resident constants ---
        ones97 = const_p.tile([97, 1], FR, tag="ones97")
        nc.sync.dma_start(out=ones97[:], in_=ones_d)
        w_sb = const_p.tile([P, IC * H], BF, tag="W")  # [p, ic*H + h]
        nc.sync.dma_start(
            out=w_sb[:].rearrange("p (ic h) -> p ic h", ic=IC),
            in_=w_d.rearrange("(ic p) h -> p ic h", p=P),
        )
        vw_sb = const_p.tile([P, HC], BF, tag="vw")    # [q, hc]
        nc.sync.dma_start(out=vw_sb[:], in_=vw_d.rearrange("(hc q) -> q hc", q=P))
        bias_sb = const_p.tile([P, HC], F32, tag="bias")
        nc.sync.dma_start(out=bias_sb[:], in_=bias_d.rearrange("(hc q) -> q hc", q=P))

        def softmax(b, s_sb):
            """exp(s - max) -> e_col (l on partitions, bf16) + 1/sum(e)."""
            n128 = lens128[b]
            n_lt = n128 // P
            negm = small_p.tile([1, 1], F32, tag="negm")
            nc.vector.reduce_max(negm[:], s_sb[0:1, :n128],
                                 axis=mybir.AxisListType.X, negate=True)
            e_sb = small_p.tile([1, n128], BF, tag="e", name=f"e{b}")
            d_sb = small_p.tile([1, 1], F32, tag="d")
            nc.scalar.activation(
                e_sb[:], s_sb[0:1, :n128], mybir.ActivationFunctionType.Exp,
                bias=negm[0:1, 0:1], scale=1.0, accum_out=d_sb[:])
            rd = small_p.tile([1, 1], F32, tag="rd")
            nc.vector.reciprocal(rd[:], d_sb[:])
            # transpose e (1, n128) -> (128, n_lt) via a DRAM round-trip
            e_dram = dram_p.tile([1, n128], BF, tag="edram", name=f"ed{b}")
            nc.sync.dma_start(out=e_dram[:], in_=e_sb[:])
            e_col = small_p.tile([P, n_lt], BF, tag="ecol", name=f"ec{b}")
            nc.sync.dma_start(
                out=e_col[:], in_=e_dram[:].rearrange("o (lt p) -> p (o lt)", p=P))
            return e_col, rd

        def pass2(b, e_col, rd):
            """r[b] = (1/d) * sum_l e_l h[b,l,:] via 4-wide col-packed PE."""
            n_lt = lens128[b] // P
            r_sb = small_p.tile([1, I], F32, tag="rsb")
            rpk = r_ps.tile([P, 256], F32, tag="rpack")
            for lt in range(n_lt):
                hn = hN_p.tile([P, I], BF, tag="hN")
                nc.sync.dma_start(
                    out=hn[:], in_=hN_d[offs128[b] + lt * P:
                                        offs128[b] + (lt + 1) * P, :])
                for j in range(4):
                    nc.tensor.matmul(
                        rpk[32 * j:32 * j + 1, :], e_col[:, lt:lt + 1],
                        hn[:, 256 * j:256 * (j + 1)],
                        start=(lt == 0), stop=(lt == n_lt - 1),
                        tile_position=(0, 32 * j))
            for j in range(4):
                nc.vector.tensor_scalar_mul(
                    r_sb[0:1, 256 * j:256 * (j + 1)], rpk[32 * j:32 * j + 1, :],
                    rd[0:1, 0:1])
            nc.sync.dma_start(out=r_d[b:b + 1, :], in_=r_sb[:])

        # ------- flat software pipeline over (batch, group) units -------
        st = {}          # per-batch state: (m_sb, s_sb)
        pend_s = []      # [(b, off, n, tanh_tiles)] awaiting packed-s emission
        pend_comb = []   # [(b, off, n, sgp)] awaiting combine emission
        pend_p2 = []     # [(b, e_col, rd)] awaiting pass2 emission

        def emit_spack(b, off, n, tanh_tiles):
            sgp = s_ps.tile([P, LG], F32, tag="spack")
            # rows other than {0,32,64,96} are read (x0.0) by the combine
            # matmul; clear them so every read byte is written by this tile
            nc.vector.memset(sgp[:, :n], 0.0)
            for hc in range(HC):
                j = hc % 4
                nc.tensor.matmul(
                    sgp[32 * j:32 * j + 1, :n], vw_sb[:, hc:hc + 1],
                    tanh_tiles[hc][:, :n],
                    start=(hc < 4), stop=(hc >= 4),
                    tile_position=(0, 32 * j))
            return sgp

        def emit_combine(b, off, n, sgp):
            m_sb, s_sb = st[b]
            part = small_p.tile([97, LG], FR, tag="spart")
            nc.vector.tensor_copy(part[:97, :n], sgp[0:97, :n])
            sg = comb_ps.tile([1, LG], F32, tag="scomb")
            nc.tensor.matmul(sg[0:1, :n], ones97[:], part[:97, :n],
                             start=True, stop=True)
            nc.vector.tensor_add(
                s_sb[0:1, off:off + n], sg[0:1, :n], m_sb[0:1, off:off + n])
            if off + n == lens[b]:          # batch fully scored
                pend_p2.append((b, *softmax(b, s_sb)))
                if len(pend_p2) > 1:
                    pass2(*pend_p2.pop(0))

        def unit(b, off, n):
            if off == 0:
                m_sb = small_p.tile([1, lens[b]], F32, tag="madd",
                                    name=f"m{b}")
                nc.sync.dma_start(out=m_sb[:],
                                  in_=madd_d[0:1, offs[b]:offs[b] + lens[b]])
                s_sb = small_p.tile([1, lens128[b]], F32, tag="s",
                                    name=f"s{b}")
                if lens128[b] > lens[b]:
                    nc.vector.memset(s_sb[0:1, lens[b]:], MASK_PAD)
                st[b] = (m_sb, s_sb)
            hT_tiles = []
            for ic in range(IC):
                t = hT_p.tile([P, LG], BF, tag="hT")
                nc.sync.dma_start(
                    out=t[:, :n],
                    in_=hT_d[ic * P:(ic + 1) * P, offs[b] + off:offs[b] + off + n])
                hT_tiles.append(t)
            tanh_tiles = []
            for hc in range(HC):
                pre = pre_ps.tile([P, LG], F32, tag="pre")
                for ic in range(IC):
                    nc.tensor.matmul(
                        pre[:, :n],
                        w_sb[:, ic * H + hc * P: ic * H + (hc + 1) * P],
                        hT_tiles[ic][:, :n],
                        start=(ic == 0), stop=(ic == IC - 1),
                    )
                th = tanh_p.tile([P, LG], BF, tag="tanh")
                nc.scalar.activation(
                    th[:, :n], pre[:, :n], mybir.ActivationFunctionType.Tanh,
                    bias=bias_sb[:, hc:hc + 1], scale=1.0)
                tanh_tiles.append(th)
            # one-unit-deferred emission hides ACT/DVE latencies inside the
            # next unit's matmul stream (across batch boundaries too)
            if pend_comb:
                emit_combine(*pend_comb.pop(0))
            if pend_s:
                ub, uo, un, tt = pend_s.pop(0)
                pend_comb.append((ub, uo, un, emit_spack(ub, uo, un, tt)))
            pend_s.append((b, off, n, tanh_tiles))

        def body():
            st.clear()
            pend_s.clear()
            pend_comb.clear()
            pend_p2.clear()
            for b in range(b_pc):
                for off, n in _groups(lens[b]):
                    unit(b, off, n)
            while pend_s or pend_comb:
                if pend_comb:
                    emit_combine(*pend_comb.pop(0))
                if pend_s:
                    ub, uo, un, tt = pend_s.pop(0)
                    pend_comb.append((ub, uo, un, emit_spack(ub, uo, un, tt)))
            while pend_p2:
                pass2(*pend_p2.pop(0))

        if static_reps:
            body()
        else:
            reps_sb = const_p.tile([1, 1], mybir.dt.int32, tag="reps")
            nc.sync.dma_start(out=reps_sb[:], in_=reps_d)
            reps_val = nc.values_load(reps_sb[0:1, 0:1], min_val=1,
                                      max_val=1 << 20,
                                      skip_runtime_bounds_check=True)
            with tc.For_i(0, reps_val, 1):
                body()

    nc.compile()
    return nc


# ---------------------------------------------------------------------------
# Host-side runtime: shard, upload, execute via PJRT (axon), gather.
# ---------------------------------------------------------------------------

class _Runtime:
    def __init__(self, nc, n_cores=N_CORES):
        from concourse import bass2jax
        from jax.sharding import Mesh, PartitionSpec, NamedSharding
        from jax.experimental.shard_map import shard_map

        bass2jax.install_neuronx_cc_hook()
        self.nc = nc
        self.n_cores = n_cores

        partition_name = (nc.partition_id_tensor.name
                          if nc.partition_id_tensor else None)
        in_names, out_names, out_avals, zero_shapes = [], [], [], []
        for alloc in nc.m.functions[0].allocations:
            if not isinstance(alloc, mybir.MemoryLocationSet):
                continue
            name = alloc.memorylocations[0].name
            if alloc.kind == "ExternalInput":
                if name != partition_name:
                    in_names.append(name)
            elif alloc.kind == "ExternalOutput":
                shape = tuple(alloc.tensor_shape)
                dtype = mybir.dt.np(alloc.dtype)
                out_names.append(name)
                out_avals.append(jax.core.ShapedArray(shape, dtype))
                zero_shapes.append((shape, dtype))
        self.in_names = list(in_names)
        self.out_names = out_names
        self.out_avals = out_avals
        self.zero_shapes = zero_shapes
        n_params = len(in_names)
        n_outs = len(out_names)
        all_names = in_names + out_names
        if partition_name is not None:
            all_names = all_names + [partition_name]

        from concourse.bass2jax import _bass_exec_p, partition_id_tensor

        def _body(*args):
            operands = list(args)
            if partition_name is not None:
                operands.append(partition_id_tensor())
            outs = _bass_exec_p.bind(
                *operands,
                out_avals=tuple(out_avals),
                in_names=tuple(all_names),
                out_names=tuple(out_names),
                lowering_input_output_aliases=(),
                sim_require_finite=False,
                sim_require_nnan=False,
                nc=nc,
            )
            return tuple(outs)

        devices = jax.devices()[:n_cores]
        self.mesh = Mesh(np.asarray(devices), ("core",))
        pspec = PartitionSpec("core")
        self.sharding = NamedSharding(self.mesh, pspec)
        donate = tuple(range(n_params, n_params + n_outs))
        self.fn = jax.jit(
            shard_map(_body, mesh=self.mesh,
                      in_specs=(pspec,) * (n_params + n_outs),
                      out_specs=(pspec,) * n_outs,
                      check_rep=False),
            donate_argnums=donate, keep_unused=True)

    def put_inputs(self, in_maps):
        concat = [
            np.concatenate([np.asarray(m[name]) for m in in_maps], axis=0)
            for name in self.in_names
        ]
        return [jax.device_put(a, self.sharding) for a in concat]

    def run(self, dev_inputs):
        zeros = [
            jax.device_put(np.zeros((self.n_cores * s[0], *s[1:]), dt), self.sharding)
            for s, dt in self.zero_shapes
        ]
        outs = self.fn(*dev_inputs, *zeros)
        jax.block_until_ready(outs)
        return outs

    def gather(self, outs):
        res = []
        for c in range(self.n_cores):
            d = {}
            for i, name in enumerate(self.out_names):
                d[name] = np.asarray(outs[i]).reshape(
                    self.n_cores, *self.out_avals[i].shape)[c]
            res.append(d)
        return res


_CACHE = {}


def _get_runtime(slot_lens=None):
    if slot_lens is None:
        # test-harness convenience: return the most recently built runtime
        assert _CACHE, "call kernel()/prep first"
        return next(iter(_CACHE.values()))
    key = tuple(slot_lens)
    if key not in _CACHE:
        nc = build_module(key)
        _CACHE[key] = _Runtime(nc)
    return _CACHE[key]


def _plan(mask):
    """Count-sorted slot assignment: rank 8j+c -> core c slot j."""
    sels = []
    for b in range(mask.shape[0]):
        sel = np.nonzero(mask[b])[0]
        if sel.size == 0:
            # fully masked: softmax(s - 1e4) == softmax(s); keep all rows
            sel = np.arange(mask.shape[1])
        sels.append(sel)
    counts = np.array([s.size for s in sels])
    order = np.argsort(-counts, kind="stable")
    slot_lens = tuple(
        max(GRAN, _ceil(int(counts[order[j * N_CORES]]), GRAN))
        for j in range(B_PC))
    return sels, order, slot_lens


def prep_in_maps(h, mask, ln_w, ln_b, v_w, vq, reps=1):
    """Host-side preprocessing + sharding into per-core input maps."""
    h = np.asarray(h, dtype=np.float32)
    mask = np.asarray(mask)
    ln_w = np.asarray(ln_w, dtype=np.float32)
    ln_b = np.asarray(ln_b, dtype=np.float32)
    v_w = np.asarray(v_w, dtype=np.float32)
    vq = np.asarray(vq, dtype=np.float32)

    sels, order, slot_lens = _plan(mask)
    lens128 = [_ceil(x, P) for x in slot_lens]
    ltot, ltot128 = sum(slot_lens), sum(lens128)

    w_hT = np.ascontiguousarray(ln_w[:, :I].T).astype(NPBF)   # (I, H)
    bias = (ln_b + ln_w[:, I:] @ vq).astype(np.float32)       # (H,)
    vw_bf = v_w.astype(NPBF)
    ones97 = np.zeros((97, 1), np.float32)
    ones97[::32] = 1.0

    in_maps = []
    for c in range(N_CORES):
        hT = np.zeros((I, ltot), NPBF)
        hN = np.zeros((ltot128, I), NPBF)
        m_add = np.full((1, ltot), MASK_PAD, np.float32)
        o = o128 = 0
        for j in range(B_PC):
            b = order[j * N_CORES + c]
            n = sels[b].size
            hb = h[b][sels[b]].astype(NPBF)        # (n, I)
            hT[:, o:o + n] = hb.T
            hN[o128:o128 + n] = hb
            m_add[0, o:o + n] = 0.0
            o += slot_lens[j]
            o128 += lens128[j]
        in_maps.append({
            "hT": hT,
            "hN": hN,
            "w_hT": w_hT,
            "v_w": vw_bf,
            "bias": bias,
            "m_add": m_add,
            "ones97": ones97,
            "reps": np.full((1, 1), reps, np.int32),
        })
    return in_maps, (slot_lens, order)


def kernel(h, mask, ln_w, ln_b, v_w, vq):
    in_maps, (slot_lens, order) = prep_in_maps(h, mask, ln_w, ln_b, v_w, vq)
    rt = _get_runtime(slot_lens)
    last_err = None
    for attempt in range(3):
        try:
            dev = rt.put_inputs(in_maps)
            outs = rt.run(dev)
            res = rt.gather(outs)
            full = np.zeros((B_FULL, I), np.float32)
            for c in range(N_CORES):
                for j in range(B_PC):
                    full[order[j * N_CORES + c]] = res[c]["r"][j]
            return full
        except Exception as e:  # transient device-unrecoverable flakes
            last_err = e
            _CACHE.pop(tuple(slot_lens), None)
            jax.clear_caches()
            rt = _get_runtime(slot_lens)
    raise last_err


# revision 6
# speedup vs baseline: 1.0740x; 1.0740x over previous
"""Trainium2 Bass kernel for nn_ConcatAttention_Param.

Reference computation (per batch b):
    pre[l,h] = sum_i h[b,l,i] * W_h[h,i] + bias[h]     (W_h = ln_w[:, :I], bias = ln_b + W_vq @ vq)
    s[l]     = tanh(pre[l,:]) @ v_w
    s       += -10000 * (~mask[b,l])
    a        = softmax(s over l)
    r[b,:]   = sum_l a[l] * h[b,l,:]

Key optimizations over a dense implementation:
  * Mask compaction: masked positions get s-10000, whose exp underflows to
    exactly 0 in fp32, so they contribute nothing to the softmax or to r.
    The host gathers only the unmasked rows of h (~half of L). This halves
    PE work, DMA traffic and tanh work. A fully-masked batch keeps all rows
    with no mask-add (softmax(s-1e4) == softmax(s)).
  * Count-sorted slot assignment: batches sorted by unmasked count, rank
    8j+c -> core c slot j, so slot j's padded length is the octile max
    (32-elem granularity) instead of the global max.
  * bf16 operands on the PE (measured end-to-end rel err ~3e-3 vs 2e-2
    budget). bf16 enables fast-weight-load and tile_position col-packing,
    both rejected for fp32r. HW microbench: LDWEIGHTS is fully hidden, and
    4-wide col-packed M=1 matmuls run concurrently (70.9 ns/MM vs 264).
  * Score dots (M=1) run 4-wide col-packed, combined with a ones matmul.
  * Pass 2 (r = e @ h) runs 4-wide col-packed at N=256.
  * Flat software pipeline over (batch, group) units: packed-score and
    combine emission are deferred one unit (hiding ACT/DVE latency inside
    the next unit's matmul stream, across batch boundaries too); pass2(b)
    is emitted one batch late so the PE never waits on the softmax chain.

Data-parallel over batch: 4 batches per core x 8 cores.
"""

from contextlib import ExitStack

import numpy as np
import ml_dtypes

import jax


import concourse.bass as bass
import concourse.tile as tile
from concourse import bacc, mybir

# Problem constants (hardcoded per contract; kernel.py may not read spec.json)
B_FULL = 32
L = 2048
I = 1024
H = 1024
N_CORES = 8
B_PC = B_FULL // N_CORES  # batches per core

LG = 512            # max l-group (moving-operand columns per matmul)
P = 128             # partitions
IC = I // P         # i chunks
HC = H // P         # h' chunks
FR = mybir.dt.float32r
F32 = mybir.dt.float32
BF = mybir.dt.bfloat16
NPBF = ml_dtypes.bfloat16

MASK_PAD = -30000.0
GRAN = 32           # score-region padding granularity


def _groups(n):
    """Split n into balanced column groups (<=512, mult of GRAN).

    Balanced (e.g. 1088 -> 384+352+352, not 512+512+64): small-N matmuls
    pay a ~60-cycle issue/drain floor, so a tiny remainder group costs far
    more per column than spreading the columns across the wide groups.
    """
    k = (n + LG - 1) // LG
    base = n // k // GRAN * GRAN
    nbig = (n - base * k) // GRAN
    offs = []
    off = 0
    for i in range(k):
        g = base + (GRAN if i < nbig else 0)
        offs.append((off, g))
        off += g
    assert off == n
    return offs


def _ceil(x, m):
    return (x + m - 1) // m * m


def build_module(slot_lens, b_pc: int = B_PC, static_reps: bool = False):
    """Build the per-core Bass module (same program on every core).

    slot_lens: per-batch-slot compacted lengths (mult of GRAN). The score
    region of slot j spans lens[j] columns; the softmax/pass-2 region is
    padded to a multiple of 128.
    """
    lens = list(slot_lens)
    assert len(lens) == b_pc and all(x % GRAN == 0 for x in lens)
    lens128 = [_ceil(x, P) for x in lens]
    offs = np.cumsum([0] + lens).tolist()       # hT / m_add column offsets
    offs128 = np.cumsum([0] + lens128).tolist()  # hN row offsets
    ltot, ltot128 = offs[-1], offs128[-1]

    nc = bacc.Bacc("TRN2", target_bir_lowering=False, debug=False,
                   enable_asserts=False, num_devices=N_CORES)

    hT_d = nc.dram_tensor("hT", (I, ltot), BF, kind="ExternalInput").ap()
    hN_d = nc.dram_tensor("hN", (ltot128, I), BF, kind="ExternalInput").ap()
    w_d = nc.dram_tensor("w_hT", (I, H), BF, kind="ExternalInput").ap()
    vw_d = nc.dram_tensor("v_w", (H,), BF, kind="ExternalInput").ap()
    bias_d = nc.dram_tensor("bias", (H,), F32, kind="ExternalInput").ap()
    madd_d = nc.dram_tensor("m_add", (1, ltot), F32, kind="ExternalInput").ap()
    ones_d = nc.dram_tensor("ones97", (97, 1), FR, kind="ExternalInput").ap()
    reps_d = nc.dram_tensor("reps", (1, 1), mybir.dt.int32,
                            kind="ExternalInput").ap()
    r_d = nc.dram_tensor("r", (b_pc, I), F32, kind="ExternalOutput").ap()

    with tile.TileContext(nc) as tc, ExitStack() as ctx:
        const_p = ctx.enter_context(tc.tile_pool(name="const", bufs=1))
        hT_p = ctx.enter_context(tc.tile_pool(name="hT", bufs=24))
        tanh_p = ctx.enter_context(tc.tile_pool(name="tanh", bufs=20))
        hN_p = ctx.enter_context(tc.tile_pool(name="hN", bufs=6))
        small_p = ctx.enter_context(tc.tile_pool(name="small", bufs=2))
        pre_ps = ctx.enter_context(tc.tile_pool(name="preps", bufs=3, space="PSUM"))
        s_ps = ctx.enter_context(tc.tile_pool(name="sps", bufs=2, space="PSUM"))
        comb_ps = ctx.enter_context(tc.tile_pool(name="combps", bufs=2, space="PSUM"))
        r_ps = ctx.enter_context(tc.tile_pool(name="rps", bufs=1, space="PSUM"))
        dram_p = ctx.enter_context(tc.tile_pool(name="edram", bufs=2, space="DRAM"))

        # --- resident constants ---
        ones97 = const_p.tile([97, 1], FR, tag="ones97")
        nc.sync.dma_start(out=ones97[:], in_=ones_d)
        w_sb = const_p.tile([P, IC * H], BF, tag="W")  # [p, ic*H + h]
        nc.sync.dma_start(
            out=w_sb[:].rearrange("p (ic h) -> p ic h", ic=IC),
            in_=w_d.rearrange("(ic p) h -> p ic h", p=P),
        )
        vw_sb = const_p.tile([P, HC], BF, tag="vw")    # [q, hc]
        nc.sync.dma_start(out=vw_sb[:], in_=vw_d.rearrange("(hc q) -> q hc", q=P))
        bias_sb = const_p.tile([P, HC], F32, tag="bias")
        nc.sync.dma_start(out=bias_sb[:], in_=bias_d.rearrange("(hc q) -> q hc", q=P))

        def softmax(b, s_sb):
            """exp(s - max) -> e_col (l on partitions, bf16) + 1/sum(e)."""
            n128 = lens128[b]
            n_lt = n128 // P
            negm = small_p.tile([1, 1], F32, tag="negm")
            nc.vector.reduce_max(negm[:], s_sb[0:1, :n128],
                                 axis=mybir.AxisListType.X, negate=True)
            e_sb = small_p.tile([1, n128], BF, tag="e", name=f"e{b}")
            d_sb = small_p.tile([1, 1], F32, tag="d")
            nc.scalar.activation(
                e_sb[:], s_sb[0:1, :n128], mybir.ActivationFunctionType.Exp,
                bias=negm[0:1, 0:1], scale=1.0, accum_out=d_sb[:])
            rd = small_p.tile([1, 1], F32, tag="rd")
            nc.vector.reciprocal(rd[:], d_sb[:])
            # transpose e (1, n128) -> (128, n_lt) via a DRAM round-trip
            e_dram = dram_p.tile([1, n128], BF, tag="edram", name=f"ed{b}")
            nc.sync.dma_start(out=e_dram[:], in_=e_sb[:])
            e_col = small_p.tile([P, n_lt], BF, tag="ecol", name=f"ec{b}")
            nc.sync.dma_start(
                out=e_col[:], in_=e_dram[:].rearrange("o (lt p) -> p (o lt)", p=P))
            return e_col, rd

        def pass2(b, e_col, rd):
            """r[b] = (1/d) * sum_l e_l h[b,l,:] via 4-wide col-packed PE."""
            n_lt = lens128[b] // P
            r_sb = small_p.tile([1, I], F32, tag="rsb")
            rpk = r_ps.tile([P, 256], F32, tag="rpack")
            for lt in range(n_lt):
                hn = hN_p.tile([P, I], BF, tag="hN")
                nc.sync.dma_start(
                    out=hn[:], in_=hN_d[offs128[b] + lt * P:
                                        offs128[b] + (lt + 1) * P, :])
                for j in range(4):
                    nc.tensor.matmul(
                        rpk[32 * j:32 * j + 1, :], e_col[:, lt:lt + 1],
                        hn[:, 256 * j:256 * (j + 1)],
                        start=(lt == 0), stop=(lt == n_lt - 1),
                        tile_position=(0, 32 * j))
            for j in range(4):
                nc.vector.tensor_scalar_mul(
                    r_sb[0:1, 256 * j:256 * (j + 1)], rpk[32 * j:32 * j + 1, :],
                    rd[0:1, 0:1])
            nc.sync.dma_start(out=r_d[b:b + 1, :], in_=r_sb[:])

        # ------- flat software pipeline over (batch, group) units -------
        st = {}          # per-batch state: (m_sb, s_sb)
        pend_s = []      # [(b, off, n, tanh_tiles)] awaiting packed-s emission
        pend_comb = []   # [(b, off, n, sgp)] awaiting combine emission
        pend_p2 = []     # [(b, e_col, rd)] awaiting pass2 emission

        def emit_spack(b, off, n, tanh_tiles):
            sgp = s_ps.tile([P, LG], F32, tag="spack")
            # rows other than {0,32,64,96} are read (x0.0) by the combine
            # matmul; clear them so every read byte is written by this tile
            nc.vector.memset(sgp[:, :n], 0.0)
            for hc in range(HC):
                j = hc % 4
                nc.tensor.matmul(
                    sgp[32 * j:32 * j + 1, :n], vw_sb[:, hc:hc + 1],
                    tanh_tiles[hc][:, :n],
                    start=(hc < 4), stop=(hc >= 4),
                    tile_position=(0, 32 * j))
            return sgp

        def emit_combine(b, off, n, sgp):
            m_sb, s_sb = st[b]
            part = small_p.tile([97, LG], FR, tag="spart")
            nc.vector.tensor_copy(part[:97, :n], sgp[0:97, :n])
            sg = comb_ps.tile([1, LG], F32, tag="scomb")
            nc.tensor.matmul(sg[0:1, :n], ones97[:], part[:97, :n],
                             start=True, stop=True)
            nc.vector.tensor_add(
                s_sb[0:1, off:off + n], sg[0:1, :n], m_sb[0:1, off:off + n])
            if off + n == lens[b]:          # batch fully scored
                pend_p2.append((b, *softmax(b, s_sb)))
                if len(pend_p2) > 1:
                    pass2(*pend_p2.pop(0))

        def unit(b, off, n):
            if off == 0:
                m_sb = small_p.tile([1, lens[b]], F32, tag="madd",
                                    name=f"m{b}")
                nc.sync.dma_start(out=m_sb[:],
                                  in_=madd_d[0:1, offs[b]:offs[b] + lens[b]])
                s_sb = small_p.tile([1, lens128[b]], F32, tag="s",
                                    name=f"s{b}")
                if lens128[b] > lens[b]:
                    nc.vector.memset(s_sb[0:1, lens[b]:], MASK_PAD)
                st[b] = (m_sb, s_sb)
            hT_tiles = []
            for ic in range(IC):
                t = hT_p.tile([P, LG], BF, tag="hT")
                nc.sync.dma_start(
                    out=t[:, :n],
                    in_=hT_d[ic * P:(ic + 1) * P, offs[b] + off:offs[b] + off + n])
                hT_tiles.append(t)
            tanh_tiles = []
            for hc in range(HC):
                pre = pre_ps.tile([P, LG], F32, tag="pre")
                for ic in range(IC):
                    nc.tensor.matmul(
                        pre[:, :n],
                        w_sb[:, ic * H + hc * P: ic * H + (hc + 1) * P],
                        hT_tiles[ic][:, :n],
                        start=(ic == 0), stop=(ic == IC - 1),
                    )
                th = tanh_p.tile([P, LG], BF, tag="tanh")
                nc.scalar.activation(
                    th[:, :n], pre[:, :n], mybir.ActivationFunctionType.Tanh,
                    bias=bias_sb[:, hc:hc + 1], scale=1.0)
                tanh_tiles.append(th)
            # one-unit-deferred emission hides ACT/DVE latencies inside the
            # next unit's matmul stream (across batch boundaries too)
            if pend_comb:
                emit_combine(*pend_comb.pop(0))
            if pend_s:
                ub, uo, un, tt = pend_s.pop(0)
                pend_comb.append((ub, uo, un, emit_spack(ub, uo, un, tt)))
            pend_s.append((b, off, n, tanh_tiles))

        def body():
            st.clear()
            pend_s.clear()
            pend_comb.clear()
            pend_p2.clear()
            for b in range(b_pc):
                for off, n in _groups(lens[b]):
                    unit(b, off, n)
            while pend_s or pend_comb:
                if pend_comb:
                    emit_combine(*pend_comb.pop(0))
                if pend_s:
                    ub, uo, un, tt = pend_s.pop(0)
                    pend_comb.append((ub, uo, un, emit_spack(ub, uo, un, tt)))
            while pend_p2:
                pass2(*pend_p2.pop(0))

        if static_reps:
            body()
        else:
            reps_sb = const_p.tile([1, 1], mybir.dt.int32, tag="reps")
            nc.sync.dma_start(out=reps_sb[:], in_=reps_d)
            reps_val = nc.values_load(reps_sb[0:1, 0:1], min_val=1,
                                      max_val=1 << 20,
                                      skip_runtime_bounds_check=True)
            with tc.For_i(0, reps_val, 1):
                body()

    nc.compile()
    return nc


# ---------------------------------------------------------------------------
# Host-side runtime: shard, upload, execute via PJRT (axon), gather.
# ---------------------------------------------------------------------------

class _Runtime:
    def __init__(self, nc, n_cores=N_CORES):
        from concourse import bass2jax
        from jax.sharding import Mesh, PartitionSpec, NamedSharding
        from jax.experimental.shard_map import shard_map

        bass2jax.install_neuronx_cc_hook()
        self.nc = nc
        self.n_cores = n_cores

        partition_name = (nc.partition_id_tensor.name
                          if nc.partition_id_tensor else None)
        in_names, out_names, out_avals, zero_shapes = [], [], [], []
        for alloc in nc.m.functions[0].allocations:
            if not isinstance(alloc, mybir.MemoryLocationSet):
                continue
            name = alloc.memorylocations[0].name
            if alloc.kind == "ExternalInput":
                if name != partition_name:
                    in_names.append(name)
            elif alloc.kind == "ExternalOutput":
                shape = tuple(alloc.tensor_shape)
                dtype = mybir.dt.np(alloc.dtype)
                out_names.append(name)
                out_avals.append(jax.core.ShapedArray(shape, dtype))
                zero_shapes.append((shape, dtype))
        self.in_names = list(in_names)
        self.out_names = out_names
        self.out_avals = out_avals
        self.zero_shapes = zero_shapes
        n_params = len(in_names)
        n_outs = len(out_names)
        all_names = in_names + out_names
        if partition_name is not None:
            all_names = all_names + [partition_name]

        from concourse.bass2jax import _bass_exec_p, partition_id_tensor

        def _body(*args):
            operands = list(args)
            if partition_name is not None:
                operands.append(partition_id_tensor())
            outs = _bass_exec_p.bind(
                *operands,
                out_avals=tuple(out_avals),
                in_names=tuple(all_names),
                out_names=tuple(out_names),
                lowering_input_output_aliases=(),
                sim_require_finite=False,
                sim_require_nnan=False,
                nc=nc,
            )
            return tuple(outs)

        devices = jax.devices()[:n_cores]
        self.mesh = Mesh(np.asarray(devices), ("core",))
        pspec = PartitionSpec("core")
        self.sharding = NamedSharding(self.mesh, pspec)
        donate = tuple(range(n_params, n_params + n_outs))
        self.fn = jax.jit(
            shard_map(_body, mesh=self.mesh,
                      in_specs=(pspec,) * (n_params + n_outs),
                      out_specs=(pspec,) * n_outs,
                      check_rep=False),
            donate_argnums=donate, keep_unused=True)

    def put_inputs(self, in_maps):
        concat = [
            np.concatenate([np.asarray(m[name]) for m in in_maps], axis=0)
            for name in self.in_names
        ]
        return [jax.device_put(a, self.sharding) for a in concat]

    def run(self, dev_inputs):
        zeros = [
            jax.device_put(np.zeros((self.n_cores * s[0], *s[1:]), dt), self.sharding)
            for s, dt in self.zero_shapes
        ]
        outs = self.fn(*dev_inputs, *zeros)
        jax.block_until_ready(outs)
        return outs

    def gather(self, outs):
        res = []
        for c in range(self.n_cores):
            d = {}
            for i, name in enumerate(self.out_names):
                d[name] = np.asarray(outs[i]).reshape(
                    self.n_cores, *self.out_avals[i].shape)[c]
            res.append(d)
        return res


_CACHE = {}


def _get_runtime(slot_lens=None):
    if slot_lens is None:
        # test-harness convenience: return the most recently built runtime
        assert _CACHE, "call kernel()/prep first"
        return next(iter(_CACHE.values()))
    key = tuple(slot_lens)
    if key not in _CACHE:
        nc = build_module(key)
        _CACHE[key] = _Runtime(nc)
    return _CACHE[key]


def _plan(mask):
    """Count-sorted slot assignment: rank 8j+c -> core c slot j."""
    sels = []
    for b in range(mask.shape[0]):
        sel = np.nonzero(mask[b])[0]
        if sel.size == 0:
            # fully masked: softmax(s - 1e4) == softmax(s); keep all rows
            sel = np.arange(mask.shape[1])
        sels.append(sel)
    counts = np.array([s.size for s in sels])
    order = np.argsort(-counts, kind="stable")
    slot_lens = tuple(
        max(GRAN, _ceil(int(counts[order[j * N_CORES]]), GRAN))
        for j in range(B_PC))
    return sels, order, slot_lens


def prep_in_maps(h, mask, ln_w, ln_b, v_w, vq, reps=1):
    """Host-side preprocessing + sharding into per-core input maps."""
    h = np.asarray(h, dtype=np.float32)
    mask = np.asarray(mask)
    ln_w = np.asarray(ln_w, dtype=np.float32)
    ln_b = np.asarray(ln_b, dtype=np.float32)
    v_w = np.asarray(v_w, dtype=np.float32)
    vq = np.asarray(vq, dtype=np.float32)

    sels, order, slot_lens = _plan(mask)
    lens128 = [_ceil(x, P) for x in slot_lens]
    ltot, ltot128 = sum(slot_lens), sum(lens128)

    w_hT = np.ascontiguousarray(ln_w[:, :I].T).astype(NPBF)   # (I, H)
    bias = (ln_b + ln_w[:, I:] @ vq).astype(np.float32)       # (H,)
    vw_bf = v_w.astype(NPBF)
    ones97 = np.zeros((97, 1), np.float32)
    ones97[::32] = 1.0

    in_maps = []
    for c in range(N_CORES):
        hT = np.zeros((I, ltot), NPBF)
        hN = np.zeros((ltot128, I), NPBF)
        m_add = np.full((1, ltot), MASK_PAD, np.float32)
        o = o128 = 0
        for j in range(B_PC):
            b = order[j * N_CORES + c]
            n = sels[b].size
            hb = h[b][sels[b]].astype(NPBF)        # (n, I)
            hT[:, o:o + n] = hb.T
            hN[o128:o128 + n] = hb
            m_add[0, o:o + n] = 0.0
            o += slot_lens[j]
            o128 += lens128[j]
        in_maps.append({
            "hT": hT,
            "hN": hN,
            "w_hT": w_hT,
            "v_w": vw_bf,
            "bias": bias,
            "m_add": m_add,
            "ones97": ones97,
            "reps": np.full((1, 1), reps, np.int32),
        })
    return in_maps, (slot_lens, order)


def kernel(h, mask, ln_w, ln_b, v_w, vq):
    in_maps, (slot_lens, order) = prep_in_maps(h, mask, ln_w, ln_b, v_w, vq)
    rt = _get_runtime(slot_lens)
    last_err = None
    for attempt in range(3):
        try:
            dev = rt.put_inputs(in_maps)
            outs = rt.run(dev)
            res = rt.gather(outs)
            full = np.zeros((B_FULL, I), np.float32)
            for c in range(N_CORES):
                for j in range(B_PC):
                    full[order[j * N_CORES + c]] = res[c]["r"][j]
            return full
        except Exception as e:  # transient device-unrecoverable flakes
            last_err = e
            _CACHE.pop(tuple(slot_lens), None)
            jax.clear_caches()
            rt = _get_runtime(slot_lens)
    raise last_err
